# revision 1
# baseline (speedup 1.0000x reference)
"""DeepseekMoE kernel for 8 Trainium2 NeuronCores.

Strategy (expert-parallel + data-parallel shared experts):
  - Host computes the router (gate matmul, softmax, top-2) in numpy and
    gathers each expert's tokens (classic MoE dispatch, done host-side as
    part of sharding).
  - Core c runs routed expert c's FFN over its gathered tokens (padded to
    a common Cpad so all 8 cores run the same SPMD program), scaling the
    output by the combine weights on-device (DVE).
  - Shared experts' weights are replicated; each core runs them over a
    distinct 512-token slice of the batch (data-parallel).
  - All matmuls run in fp16 (1 cycle/row on the PE, same rate as bf16
    but with 10-bit mantissa -> ~8x less rounding error; fp32 is 4x
    slower) with fp32 PSUM accumulation; GELU (exact/erf) on the ACT
    engine reading PSUM directly.
  - Layout is fully transposed (features on partitions, tokens on the
    free dim) so the two FFN matmuls chain with no on-chip transposes.
    Host pre-packs every operand into [128, *] row-major blocks so each
    DMA is a contiguous >=512 KB transfer (HWDGE generation overhead is
    ~625 ns/DMA, so small DMAs cap effective HBM bandwidth).
  - The f-loop is software-pipelined (lookahead 2) across chunks and
    phases so the PE never stalls on ACT; output DMAs ride the SWDGE
    (gpsimd) path so they are not head-of-line blocked behind the
    input preload on the HWDGE queues.
  - Host scatters per-expert outputs back (each token appears in exactly
    K=2 experts) and adds the (zero, but handled exactly) output biases.
"""

import numpy as np
import ml_dtypes

import concourse.bass as bass
import concourse.tile as tile
import concourse.mybir as mybir
from concourse import bacc
from concourse.bass_utils import run_bass_kernel_spmd

B, S, D, F, E, NS, K = 2, 2048, 512, 2048, 8, 2, 2
T = B * S
N_CORES = 8
TS = T // N_CORES          # shared-expert tokens per core
FS = NS * F                # concatenated shared FFN width
CHUNK = 512                # token chunk (= max fp32 PSUM bank free dim)
KD = D // 128              # 4  k-tiles over D
FR = F // 128              # 16 f-tiles routed
FShared = FS // 128        # 32 f-tiles shared
DD = D // 128              # 4  output d-tiles
WG = 4                     # f-tiles per w-DMA group (512 KB transfers)

BF16 = mybir.dt.float16
F32 = mybir.dt.float32
np_bf16 = np.float16

_GELU = mybir.ActivationFunctionType.Gelu

_cache: dict = {}


def _routed_sizes(cpad):
    """Token-chunk sizes for the routed phase: a small first chunk (fast PE
    start — less DMA to wait for), 512s in the middle, and a smallish final
    chunk (short drain tail). No chunk below 256 — small-N matmuls go
    LDWEIGHTS-bound on real hardware."""
    if cpad <= CHUNK:
        return [cpad]
    head = cpad - 768
    if 256 <= head <= CHUNK:                # the realistic range
        return [head, CHUNK, 256]
    if cpad < 1024:
        mid = cpad - 512
        return [256] + ([mid] if mid else []) + [256]
    sizes, rem = [256], cpad - 768          # reserve two 256 tail chunks
    while rem > CHUNK:
        take = CHUNK if rem - CHUNK >= 256 else rem - 256
        sizes.append(take)
        rem -= take
    sizes.append(rem)
    return sizes + [256, 256]


def _shared_sizes(ts):
    """Shared-expert chunk sizes; ends on a 256 chunk for a short tail."""
    return [ts] if ts <= 256 else [ts - 256, 256]


def _chunk_offsets(total, sizes=None):
    """(start, size) pairs; default uniform CHUNK split."""
    if sizes is None:
        sizes = [min(CHUNK, total - c0) for c0 in range(0, total, CHUNK)]
    out, c0 = [], 0
    for s in sizes:
        out.append((c0, s))
        c0 += s
    return out


def _build(cpad: int):
    nc = bacc.Bacc("TRN2", debug=False)

    xg = nc.dram_tensor("xg", [128, KD * cpad], BF16, kind="ExternalInput")
    cwb = nc.dram_tensor("cwb", [128, cpad], F32, kind="ExternalInput")
    rw1t = nc.dram_tensor("rw1t", [128, KD * F], BF16, kind="ExternalInput")
    rw2t = nc.dram_tensor("rw2t", [128, FR * D], BF16, kind="ExternalInput")
    rb1 = nc.dram_tensor("rb1", [128, FR], F32, kind="ExternalInput")
    xs = nc.dram_tensor("xs", [128, KD * TS], BF16, kind="ExternalInput")
    sw1t = nc.dram_tensor("sw1t", [128, KD * FS], BF16, kind="ExternalInput")
    sw2t = nc.dram_tensor("sw2t", [128, FShared * D], BF16, kind="ExternalInput")
    sb1 = nc.dram_tensor("sb1", [128, FShared], F32, kind="ExternalInput")
    yr = nc.dram_tensor("yr", [D, cpad], F32, kind="ExternalOutput")
    ys = nc.dram_tensor("ys", [D, TS], BF16, kind="ExternalOutput")

    with tile.TileContext(nc) as tc:
        with (
            tc.tile_pool(name="wts", bufs=1) as wts,
            tc.tile_pool(name="acts", bufs=1) as acts,
            tc.tile_pool(name="hp", bufs=4) as hp,
            tc.tile_pool(name="op", bufs=3) as op,
            tc.tile_pool(name="ps1", bufs=4, space="PSUM") as ps1,
            tc.tile_pool(name="ps2", bufs=1, space="PSUM") as ps2,
        ):
            # ---- t=0 warmup while the first DMAs are in flight: trigger the
            # GELU ACT-table load now (it costs ~1.3 us on first use), and run
            # dummy matmuls so the PE p-state/HAM is at full clock when the
            # first real matmul issues ----
            warm = wts.tile([128, 512], BF16, name="warm_in")
            nc.vector.memset(warm[:], 0.0)
            wb = wts.tile([128, 1], F32, name="warm_b")
            nc.vector.memset(wb[:], 0.0)
            wh = hp.tile([128, 512], BF16, name="wh")
            nc.scalar.activation(wh[:], warm[:, 0:512], _GELU, bias=wb[:])
            wp = ps1.tile([128, 512], F32, tag="p1", name="warmp")
            for _ in range(6):
                nc.tensor.matmul(wp[:], warm[:, 0:128], warm[:], start=True, stop=True)

            # ---- resident SBUF images of all inputs ----
            xg_sb = acts.tile([128, KD * cpad], BF16, name="xg_sb")
            rw1_sb = wts.tile([128, KD * F], BF16, name="rw1_sb")
            rw2_sb = wts.tile([128, FR * D], BF16, name="rw2_sb")
            rb1_sb = wts.tile([128, FR], F32, name="rb1_sb")
            cw_sb = acts.tile([128, cpad], F32, name="cw_sb")
            xs_sb = acts.tile([128, KD * TS], BF16, name="xs_sb")
            sw1_sb = wts.tile([128, KD * FS], BF16, name="sw1_sb")
            sw2_sb = wts.tile([128, FShared * D], BF16, name="sw2_sb")
            sb1_sb = wts.tile([128, FShared], F32, name="sb1_sb")

            def col_dma(dst, src, lo, hi):
                nc.sync.dma_start(dst[:, lo:hi], src.ap()[:, lo:hi])

            def w1_group_dma(dst, src, f_lo, f_hi):
                # f-columns [f_lo*128, f_hi*128) for every k-block
                d4 = dst.rearrange("p (k f) -> p k f", k=KD)
                s4 = src.ap().rearrange("p (k f) -> p k f", k=KD)
                nc.sync.dma_start(d4[:, :, f_lo * 128:f_hi * 128],
                                  s4[:, :, f_lo * 128:f_hi * 128])

            # consumption-ordered preload (HWDGE)
            chunks_r = _chunk_offsets(cpad, _routed_sizes(cpad))
            c0, cs = chunks_r[0]
            xoff = [0]
            for _, s in chunks_r:
                xoff.append(xoff[-1] + KD * s)
            # chunk-0 tokens ride SWDGE so their descriptor generation runs in
            # parallel with rw1's HWDGE generation (shorter startup chain)
            nc.gpsimd.dma_start(xg_sb[:, 0:xoff[1]], xg.ap()[:, 0:xoff[1]])
            w1_group_dma(rw1_sb, rw1t, 0, 2)                    # rw1 f0..f1
            nc.sync.dma_start(rb1_sb[:], rb1.ap())
            col_dma(rw2_sb, rw2t, 0, WG * D)                    # rw2 f0..f3
            w1_group_dma(rw1_sb, rw1t, 2, 4)
            for g in range(1, FR // WG):
                w1_group_dma(rw1_sb, rw1t, g * WG, (g + 1) * WG)
                col_dma(rw2_sb, rw2t, g * WG * D, (g + 1) * WG * D)
            col_dma(xg_sb, xg, xoff[1], xoff[-1])               # remaining tokens
            nc.sync.dma_start(cw_sb[:], cwb.ap())
            nc.sync.dma_start(xs_sb[:], xs.ap())
            nc.sync.dma_start(sb1_sb[:], sb1.ap())
            for g in range(FShared // (2 * WG)):                # 1 MB transfers
                w1_group_dma(sw1_sb, sw1t, g * 2 * WG, (g + 1) * 2 * WG)
                col_dma(sw2_sb, sw2t, g * 2 * WG * D, (g + 1) * 2 * WG * D)

            # ---- chunk descriptors: small routed chunk first (fast start),
            # shared phase last, ending on a small chunk (short tail) ----
            def r_chunk(i, c0, cs):
                return dict(
                    cs=cs, c0=c0, nf=FR, cw=True, y=yr, b1=rb1_sb,
                    x=lambda k, o=xoff[i], cs=cs: xg_sb[:, o + k * cs:o + (k + 1) * cs],
                    w1=lambda k, f: rw1_sb[:, k * F + f * 128:k * F + (f + 1) * 128],
                    w2=lambda f, d: rw2_sb[:, f * D + d * 128:f * D + (d + 1) * 128],
                )

            def s_chunk(i, c0, cs):
                return dict(
                    cs=cs, c0=c0, nf=FShared, cw=False, y=ys, b1=sb1_sb,
                    x=lambda k, i=i, cs=cs: xs_sb[:, soff[i] + k * cs:soff[i] + (k + 1) * cs],
                    w1=lambda k, f: sw1_sb[:, k * FS + f * 128:k * FS + (f + 1) * 128],
                    w2=lambda f, d: sw2_sb[:, f * D + d * 128:f * D + (d + 1) * 128],
                )

            chunks_s = _chunk_offsets(TS, _shared_sizes(TS))
            soff = [0]
            for _, s in chunks_s:
                soff.append(soff[-1] + KD * s)
            routed = [r_chunk(i, c0, cs) for i, (c0, cs) in enumerate(chunks_r)]
            shared = [s_chunk(i, c0, cs) for i, (c0, cs) in enumerate(chunks_s)]
            chunks = routed + shared
            steps = [(ch, f) for ch in chunks for f in range(ch["nf"])]

            # ---- software-pipelined emission: PE issues the f-tile's
            # first-layer matmuls LOOKAHEAD steps ahead of the second-layer
            # matmuls that consume the GELU output ----
            LOOKAHEAD = 2
            h_tiles: dict = {}
            po_tiles: dict = {}
            for i in range(len(steps) + LOOKAHEAD):
                if i < len(steps):
                    ch, f = steps[i]
                    cs = ch["cs"]
                    p1 = ps1.tile([128, cs], F32, name="p1")
                    for k in range(KD):
                        nc.tensor.matmul(
                            p1[:], ch["w1"](k, f), ch["x"](k),
                            start=(k == 0), stop=(k == KD - 1),
                        )
                    h = hp.tile([128, cs], BF16, name="h")
                    nc.scalar.activation(h[:], p1[:], _GELU, bias=ch["b1"][:, f:f + 1])
                    h_tiles[i] = h
                j = i - LOOKAHEAD
                if j >= 0:
                    ch, f = steps[j]
                    cs, c0 = ch["cs"], ch["c0"]
                    if f == 0:
                        po_tiles[id(ch)] = [
                            ps2.tile([128, cs], F32, tag=f"o{d}", name=f"po{d}")
                            for d in range(DD)
                        ]
                    po = po_tiles[id(ch)]
                    h = h_tiles.pop(j)
                    for d in range(DD):
                        nc.tensor.matmul(
                            po[d][:], ch["w2"](f, d), h[:],
                            start=(f == 0), stop=(f == ch["nf"] - 1),
                        )
                    if f == ch["nf"] - 1:
                        o = op.tile([128, DD * cs], F32 if ch["cw"] else BF16,
                                    name="o")
                        last = ch is chunks[-1]
                        for d in range(DD):
                            if ch["cw"]:
                                nc.vector.tensor_mul(
                                    o[:, d * cs:(d + 1) * cs], po[d][:],
                                    cw_sb[:, c0:c0 + cs])
                            elif last and d >= 2:
                                # tail chunk: split evacuation across ACT and
                                # DVE so the final drain starts sooner
                                nc.scalar.copy(o[:, d * cs:(d + 1) * cs], po[d][:])
                            else:
                                nc.vector.tensor_copy(
                                    o[:, d * cs:(d + 1) * cs], po[d][:])
                        # one wide DMA per chunk on the SWDGE path: separate
                        # FIFO from the input preload (no head-of-line block),
                        # and one generation overhead instead of four. The
                        # final chunk rides HWDGE (lower latency; preload is
                        # long finished) to shorten the kernel tail.
                        ydst = ch["y"].ap().rearrange(
                            "(dd p) c -> p dd c", p=128)[:, :, c0:c0 + cs]
                        ysrc = o.rearrange("p (dd c) -> p dd c", dd=DD)
                        if last:
                            nc.sync.dma_start(ydst, ysrc)
                        else:
                            nc.gpsimd.dma_start(ydst, ysrc)
                        del po_tiles[id(ch)]

    nc.compile()
    return nc


def _pack_k_blocks(a2d):
    """[K*128, N] -> [128, K*N] with k-blocks along the free dim."""
    k = a2d.shape[0] // 128
    return np.ascontiguousarray(
        a2d.reshape(k, 128, -1).transpose(1, 0, 2).reshape(128, -1))


def _pack_chunked(xT, total, sizes=None):
    """[D, total] -> [128, KD*total] grouped chunk-major: for each chunk c,
    the KD k-blocks of that chunk's columns are laid out consecutively."""
    parts = []
    for c0, cs in _chunk_offsets(total, sizes):
        blk = xT[:, c0:c0 + cs]                      # [D, cs]
        parts.append(blk.reshape(KD, 128, cs).transpose(1, 0, 2).reshape(128, -1))
    return np.ascontiguousarray(np.concatenate(parts, axis=1))


def kernel(x, gate_w, gate_b, sw1, sb1, sw2, sb2, rw1, rb1, rw2, rb2):
    x = np.asarray(x, np.float32)
    gate_w = np.asarray(gate_w, np.float32)
    gate_b = np.asarray(gate_b, np.float32)
    sw1 = np.asarray(sw1, np.float32)
    sb1 = np.asarray(sb1, np.float32)
    sw2 = np.asarray(sw2, np.float32)
    sb2 = np.asarray(sb2, np.float32)
    rw1 = np.asarray(rw1, np.float32)
    rb1 = np.asarray(rb1, np.float32)
    rw2 = np.asarray(rw2, np.float32)
    rb2 = np.asarray(rb2, np.float32)

    t = x.reshape(T, D)

    # ---- router on host (part of the dispatch/sharding step) ----
    logits = t @ gate_w.T + gate_b
    m = logits.max(axis=1, keepdims=True)
    ex = np.exp(logits - m)
    probs = ex / ex.sum(axis=1, keepdims=True)
    top_i = np.argpartition(-probs, K - 1, axis=1)[:, :K]          # [T, K]

    sel = np.zeros((T, E), bool)
    sel[np.arange(T)[:, None], top_i] = True
    idxs = [np.nonzero(sel[:, e])[0] for e in range(E)]
    counts = np.array([len(i) for i in idxs])
    cpad = max(CHUNK, int(-(-counts.max() // 4) * 4))

    if cpad not in _cache:
        _cache[cpad] = _build(cpad)
    nc = _cache[cpad]

    # ---- shared-expert weights, concatenated over NS and packed ----
    sw1t = _pack_k_blocks(sw1.reshape(FS, D).T.astype(np_bf16))
    sw2t = _pack_k_blocks(sw2.transpose(0, 2, 1).reshape(FS, D).astype(np_bf16))
    sb1c = np.ascontiguousarray(sb1.reshape(FShared, 128).T)

    in_maps = []
    for c in range(N_CORES):
        idx = idxs[c]
        ce = len(idx)
        xgT = np.zeros((D, cpad), np_bf16)
        xgT[:, :ce] = t[idx].T.astype(np_bf16)
        cwb = np.zeros((128, cpad), np.float32)
        cwb[:, :ce] = probs[idx, c][None, :]
        in_maps.append({
            "xg": _pack_chunked(xgT, cpad, _routed_sizes(cpad)),
            "cwb": cwb,
            "rw1t": _pack_k_blocks(rw1[c].T.astype(np_bf16)),
            "rw2t": _pack_k_blocks(rw2[c].T.astype(np_bf16)),
            "rb1": np.ascontiguousarray(rb1[c].reshape(FR, 128).T),
            "xs": _pack_chunked(
                np.ascontiguousarray(t[c * TS:(c + 1) * TS].T.astype(np_bf16)),
                TS, _shared_sizes(TS)),
            "sw1t": sw1t,
            "sw2t": sw2t,
            "sb1": sb1c,
        })

    res = run_bass_kernel_spmd(nc, in_maps, core_ids=list(range(N_CORES)))

    # ---- combine on host ----
    out = np.empty((T, D), np.float32)
    for c in range(N_CORES):
        out[c * TS:(c + 1) * TS] = res.results[c]["ys"].T.astype(np.float32)
    for c in range(N_CORES):
        idx = idxs[c]
        out[idx] += res.results[c]["yr"][:, :len(idx)].T

    # output biases (zero in the spec, handled exactly anyway)
    if sb2.any() or rb2.any():
        cw = np.zeros((T, E), np.float32)
        np.add.at(cw, (np.arange(T)[:, None], top_i),
                  np.take_along_axis(probs, top_i, axis=1))
        out += sb2.sum(axis=0)[None, :] + cw @ rb2

    return out.reshape(B, S, D)



# revision 10
# speedup vs baseline: 1.3455x; 1.3455x over previous
"""DeepseekMoE kernel for 8 Trainium2 NeuronCores.

Strategy (expert-parallel routed + data-parallel shared, fp8 DoubleRow):
  - Host computes the router (gate matmul, softmax, top-2) in numpy and
    gathers each expert's tokens (classic MoE dispatch, done host-side as
    part of sharding).
  - Core c runs routed expert c's FFN over its gathered tokens (padded to
    a common Cpad, multiple of 16, so all 8 cores run the same SPMD
    program), scaling the output by the combine weights on-device (DVE).
  - Shared experts' weights are replicated; each core runs them over a
    distinct 512-token slice of the batch (data-parallel).
  - Matmul precision plan (PSUM always accumulates fp32):
      * routed L1/L2: fp8 e4m3 operands with perf_mode=DoubleRow
        (2 fp8 weights/PE cell -> 2 k-tiles contracted per instruction).
        The routed output is scaled by top-2 softmax weights (~0.17 norm
        share of the final output), so 1-term fp8 error (~5% rel) only
        contributes ~0.9% to the result.
      * shared L1: 3-term residual-corrected fp8 DoubleRow:
          256*(x@w1) = xh@W1h + rx16@W1h16 + xh@W1lo
        with xh=e4m3(x), rx16=e4m3(16*(x-xh)), W1h=e4m3(256*w1),
        W1h16=e4m3(16*w1), W1lo=e4m3(256*w1-W1h). Host packs all copies.
        The fp8 weight scale (256x) centers N(0, 0.02) weights in e4m3's
        normal range; it is undone by the GELU activation's input scale.
      * shared L2: plain fp16 (h written fp16 by ACT; w2 stored fp16) --
        the shared output carries ~98% of the result norm, so its h/w2
        cannot take a single-quantization fp8 error.
  - Layout is fully transposed (features on partitions, tokens on the
    free dim), k-blocks of the contraction as a middle AP dim so
    DoubleRow's [128, 2, n] operand shape falls out of a plain slice.
  - Input DMAs are a few large transfers (>=512B contiguous runs for
    full modeled DMA bandwidth), issued in consumption order and spread
    across the SP and DVE sequencers (each dma_start costs ~1.2us of
    sequencer time); outputs ride the SWDGE (gpsimd) path.
"""

import numpy as np
import ml_dtypes

import concourse.bass as bass
import concourse.tile as tile
import concourse.mybir as mybir
from concourse import bacc
from concourse.bass_utils import run_bass_kernel_spmd

B, S, D, F, E, NS, K = 2, 2048, 512, 2048, 8, 2, 2
T = B * S
N_CORES = 8
TS = T // N_CORES          # shared-expert tokens per core
FS = NS * F                # concatenated shared FFN width
CHUNK = 512                # token chunk (= max fp32 PSUM bank free dim)
KD = D // 128              # 4  k-tiles over D
FR = F // 128              # 16 f-tiles routed
FSH = FS // 128            # 32 f-tiles shared
DD = D // 128              # 4  output d-tiles
WG = 4                     # f-tiles per shared weight-group DMA

FP8 = mybir.dt.float8e4
F16 = mybir.dt.float16
F32 = mybir.dt.float32
np_f8 = ml_dtypes.float8_e4m3
np_f16 = np.float16

_GELU = mybir.ActivationFunctionType.Gelu
_DR = mybir.MatmulPerfMode.DoubleRow
WSCALE = 256.0
RSCALE = 16.0

_cache: dict = {}


def _routed_sizes(cpad):
    """Token-chunk sizes for the routed phase: small first chunk (fast PE
    start -- less DMA to wait for), 512s after. All sizes multiples of 16
    (DoubleRow AP step constraint)."""
    if cpad <= CHUNK:
        return [cpad]
    head = cpad - ((cpad - 256) // CHUNK) * CHUNK
    if head > CHUNK:
        a = head // 2 // 16 * 16
        sizes = [a, head - a]
    else:
        sizes = [head]
    sizes += [CHUNK] * ((cpad - head) // CHUNK)
    return sizes


def _chunk_offsets(total, sizes):
    out, c0 = [], 0
    for s in sizes:
        out.append((c0, s))
        c0 += s
    return out


def _build(cpad: int):
    nc = bacc.Bacc("TRN2", debug=False)

    sizes_r = _routed_sizes(cpad)
    segA = sizes_r[0]            # first chunk: own k-major segment
    segB = cpad - segA           # remaining chunks share one segment
    chunks_r = _chunk_offsets(cpad, sizes_r)

    xg = nc.dram_tensor("xg", [128, KD * cpad], FP8, kind="ExternalInput")
    cwb = nc.dram_tensor("cwb", [128, cpad], F16, kind="ExternalInput")
    rw = nc.dram_tensor("rw", [128, KD * F + FR * D], FP8, kind="ExternalInput")
    rb1 = nc.dram_tensor("rb1", [128, FR], F32, kind="ExternalInput")
    xs = nc.dram_tensor("xs", [128, 2 * KD * TS], FP8, kind="ExternalInput")
    sw1 = nc.dram_tensor("sw1", [128, 3 * KD * FS], FP8, kind="ExternalInput")
    sw2 = nc.dram_tensor("sw2", [128, FSH * D], F16, kind="ExternalInput")
    sb1 = nc.dram_tensor("sb1", [128, FSH], F32, kind="ExternalInput")
    yr = nc.dram_tensor("yr", [D, cpad], F16, kind="ExternalOutput")
    ys = nc.dram_tensor("ys", [D, TS], F16, kind="ExternalOutput")

    # sw1 free-dim layout: [group g(8)][copy t(3)][k(4)][fcol(WG*128)]
    GCOL = 3 * KD * WG * 128     # columns per group block

    with tile.TileContext(nc) as tc:
        with (
            tc.tile_pool(name="wts", bufs=1) as wts,
            tc.tile_pool(name="acts", bufs=1) as acts,
            tc.tile_pool(name="hp", bufs=4) as hp,
            tc.tile_pool(name="op", bufs=3) as op,
            tc.tile_pool(name="ps1", bufs=4, space="PSUM") as ps1,
            tc.tile_pool(name="ps2", bufs=1, space="PSUM") as ps2,
        ):
            # ---- t=0 warmup while the first DMAs are in flight: trigger the
            # GELU ACT-table load now (~1.3 us on first use), and run dummy
            # matmuls so the PE p-state is at full clock for the first real
            # matmul ----
            warm = wts.tile([128, 512], FP8, name="warm_in")
            nc.vector.memset(warm[:], 0.0)
            wb = wts.tile([128, 1], F32, name="warm_b")
            nc.vector.memset(wb[:], 0.0)
            wh = hp.tile([128, 512], FP8, name="wh")
            nc.scalar.activation(wh[:], warm[:, 0:512], _GELU, bias=wb[:])
            wp = ps1.tile([128, 512], F32, tag="p1", name="warmp")
            w3 = warm.rearrange("p (two c) -> p two c", two=2)
            for _ in range(12):
                nc.tensor.matmul(wp[:, 0:256], w3[:, :, 0:128],
                                 w3[:, :, 0:256], start=True, stop=True,
                                 perf_mode=_DR)

            # ---- resident SBUF images of all inputs ----
            xg_sb = acts.tile([128, KD * cpad], FP8, name="xg_sb")
            rw_sb = wts.tile([128, KD * F + FR * D], FP8, name="rw_sb")
            rb1_sb = wts.tile([128, FR], F32, name="rb1_sb")
            cw_sb = acts.tile([128, cpad], F16, name="cw_sb")
            xs_sb = acts.tile([128, 2 * KD * TS], FP8, name="xs_sb")
            sw1_sb = wts.tile([128, 3 * KD * FS], FP8, name="sw1_sb")
            sw2_sb = wts.tile([128, FSH * D], F16, name="sw2_sb")
            sb1_sb = wts.tile([128, FSH], F32, name="sb1_sb")

            # ---- consumption-ordered preload ----
            # chunk-0 tokens ride SWDGE so their descriptor generation runs
            # in parallel with rw's HWDGE generation (shorter startup chain)
            nc.gpsimd.dma_start(xg_sb[:, 0:KD * segA], xg.ap()[:, 0:KD * segA])
            RW1, RWQ = KD * F, (KD * F) // 4
            # routed weights in 4 interleaved f-major quarters (w1 f-quarter
            # + w2 f-quarter), SP queue; both are packed f-group-major so
            # each quarter is one contiguous transfer in consumption order
            for q in range(4):
                nc.sync.dma_start(
                    rw_sb[:, q * RWQ:(q + 1) * RWQ],
                    rw.ap()[:, q * RWQ:(q + 1) * RWQ])
                nc.sync.dma_start(
                    rw_sb[:, RW1 + q * RWQ:RW1 + (q + 1) * RWQ],
                    rw.ap()[:, RW1 + q * RWQ:RW1 + (q + 1) * RWQ])
            nc.sync.dma_start(rb1_sb[:], rb1.ap())
            nc.sync.dma_start(xg_sb[:, KD * segA:], xg.ap()[:, KD * segA:])
            nc.sync.dma_start(cw_sb[:], cwb.ap())
            # shared inputs on the SWDGE/gpsimd queue (Pool sequencer is
            # idle; SP's is busy with the routed preload and ACT's queue
            # must stay clear for the GELU stream)
            nc.gpsimd.dma_start(xs_sb[:], xs.ap())
            nc.gpsimd.dma_start(sb1_sb[:], sb1.ap())
            for g in range(FSH // WG):
                nc.gpsimd.dma_start(sw1_sb[:, g * GCOL:(g + 1) * GCOL],
                                    sw1.ap()[:, g * GCOL:(g + 1) * GCOL])
                nc.gpsimd.dma_start(
                    sw2_sb[:, g * WG * D:(g + 1) * WG * D],
                    sw2.ap()[:, g * WG * D:(g + 1) * WG * D])

            # ---- AP helpers ----
            xg3A = xg_sb[:, 0:KD * segA].rearrange("p (k c) -> p k c", k=KD)
            xg3B = xg_sb[:, KD * segA:].rearrange("p (k c) -> p k c", k=KD)
            # rw1 packed f-group-major: [g(4)][k(4)][fcol(512)]
            rw13 = rw_sb[:, 0:RW1].rearrange("p (g k f) -> p g k f", g=4, k=KD)
            rw23 = rw_sb[:, RW1:].rearrange("p (f d) -> p f d", f=FR)
            xs3 = xs_sb.rearrange("p (t k c) -> p t k c", t=2, k=KD)
            sw13 = sw1_sb.rearrange("p (g t k f) -> p g t k f", g=FSH // WG,
                                    t=3, k=KD)
            sw23 = sw2_sb.rearrange("p (f d) -> p f d", f=FSH)

            def x_route(i, kk, cs):
                if i == 0:
                    return xg3A[:, 2 * kk:2 * kk + 2, 0:cs]
                c0 = chunks_r[i][0] - segA
                return xg3B[:, 2 * kk:2 * kk + 2, c0:c0 + cs]

            # ---- chunk descriptors ----
            def r_chunk(i, c0, cs):
                return dict(
                    kind="r", cs=cs, c0=c0, nf=FR,
                    x=lambda kk, i=i, cs=cs: x_route(i, kk, cs),
                    w1=lambda kk, f: rw13[:, f // 4, 2 * kk:2 * kk + 2,
                                          (f % 4) * 128:(f % 4 + 1) * 128],
                    w2=lambda fp, d: rw23[:, 2 * fp:2 * fp + 2,
                                          d * 128:(d + 1) * 128],
                )

            def s_chunk(c0, cs):
                return dict(
                    kind="s", cs=cs, c0=c0, nf=FSH,
                    x=lambda t, kk, c0=c0, cs=cs:
                        xs3[:, t, 2 * kk:2 * kk + 2, c0:c0 + cs],
                    w1=lambda t, kk, f: sw13[:, f // WG, t, 2 * kk:2 * kk + 2,
                                             (f % WG) * 128:
                                             (f % WG + 1) * 128],
                    w2=lambda f, d: sw23[:, f, d * 128:(d + 1) * 128],
                )

            chunks = [r_chunk(i, c0, cs) for i, (c0, cs) in enumerate(chunks_r)]
            chunks += [s_chunk(c0, cs)
                       for c0, cs in _chunk_offsets(TS, [CHUNK] * (TS // CHUNK))]
            steps = [(ch, f) for ch in chunks for f in range(ch["nf"])]

            # ---- software-pipelined emission: the f-tile's first-layer
            # matmuls run LOOKAHEAD steps ahead of the second-layer matmuls
            # that consume its activation output ----
            LOOKAHEAD = 2
            h_tiles: dict = {}
            po_tiles: dict = {}
            for i in range(len(steps) + LOOKAHEAD):
                if i < len(steps):
                    ch, f = steps[i]
                    cs = ch["cs"]
                    # PSUM tiles are allocated bank-sized (512 fp32) and
                    # prefix-sliced so a partial chunk never crosses a bank
                    p1 = ps1.tile([128, CHUNK], F32, name="p1")[:, 0:cs]
                    if ch["kind"] == "r":
                        for kk in range(KD // 2):
                            nc.tensor.matmul(
                                p1[:], ch["w1"](kk, f), ch["x"](kk),
                                start=(kk == 0), stop=(kk == KD // 2 - 1),
                                perf_mode=_DR)
                        # routed h tiles come in pairs: DoubleRow L2 consumes
                        # [128, 2, cs] (two f-tiles) per instruction
                        if f % 2 == 0:
                            hpair = hp.tile([128, 2 * cs], FP8, name="h")
                            h_tiles[i] = hpair
                        else:
                            hpair = h_tiles[i - 1]
                            h_tiles[i] = hpair
                        nc.scalar.activation(
                            hpair[:, (f % 2) * cs:(f % 2 + 1) * cs], p1[:],
                            _GELU, bias=rb1_sb[:, f:f + 1], scale=1.0 / WSCALE)
                    else:
                        for t in range(3):
                            for kk in range(KD // 2):
                                nc.tensor.matmul(
                                    p1[:], ch["w1"](t, kk, f),
                                    ch["x"](0 if t != 1 else 1, kk),
                                    start=(t == 0 and kk == 0),
                                    stop=(t == 2 and kk == KD // 2 - 1),
                                    perf_mode=_DR)
                        h = hp.tile([128, cs], F16, name="hs")
                        nc.scalar.activation(h[:], p1[:], _GELU,
                                             bias=sb1_sb[:, f:f + 1],
                                             scale=1.0 / WSCALE)
                        h_tiles[i] = h
                j = i - LOOKAHEAD
                if j >= 0:
                    ch, f = steps[j]
                    cs, c0 = ch["cs"], ch["c0"]
                    routed = ch["kind"] == "r"
                    if f == 0:
                        po_tiles[id(ch)] = [
                            ps2.tile([128, CHUNK], F32, tag=f"o{d}",
                                     name=f"po{d}")[:, 0:cs]
                            for d in range(DD)
                        ]
                    po = po_tiles[id(ch)]
                    h = h_tiles.pop(j)
                    if routed:
                        if f % 2 == 1:      # consume the completed h pair
                            h3 = h.rearrange("p (two c) -> p two c", two=2)
                            for d in range(DD):
                                nc.tensor.matmul(
                                    po[d][:], ch["w2"](f // 2, d), h3[:],
                                    start=(f == 1), stop=(f == ch["nf"] - 1),
                                    perf_mode=_DR)
                    else:
                        for d in range(DD):
                            nc.tensor.matmul(
                                po[d][:], ch["w2"](f, d), h[:],
                                start=(f == 0), stop=(f == ch["nf"] - 1))
                    if f == ch["nf"] - 1:
                        o = op.tile([128, DD * cs], F16, name="o")
                        last = ch is chunks[-1]
                        for d in range(DD):
                            if routed:
                                nc.vector.tensor_mul(
                                    o[:, d * cs:(d + 1) * cs], po[d][:],
                                    cw_sb[:, c0:c0 + cs])
                            elif last and d >= 2:
                                # tail chunk: split evacuation across ACT and
                                # DVE so the final drain starts sooner
                                nc.scalar.copy(o[:, d * cs:(d + 1) * cs],
                                               po[d][:])
                            else:
                                nc.vector.tensor_copy(
                                    o[:, d * cs:(d + 1) * cs], po[d][:])
                        # one wide DMA per chunk; SWDGE path (separate FIFO
                        # from the input preload). The final chunk rides
                        # HWDGE (lower latency; preload long finished).
                        ydst = (yr if routed else ys).ap().rearrange(
                            "(dd p) c -> p dd c", p=128)[:, :, c0:c0 + cs]
                        ysrc = o.rearrange("p (dd c) -> p dd c", dd=DD)
                        if last:
                            nc.sync.dma_start(ydst, ysrc)
                        else:
                            nc.gpsimd.dma_start(ydst, ysrc)
                        del po_tiles[id(ch)]

    nc.compile()
    return nc


def _q8(a):
    return np.asarray(a, np.float32).astype(np_f8)


def _pack_k(a2d):
    """[K*128, N] -> [128, K*N] with k-blocks along the free dim."""
    k = a2d.shape[0] // 128
    return np.ascontiguousarray(
        a2d.reshape(k, 128, -1).transpose(1, 0, 2).reshape(128, -1))


def kernel(x, gate_w, gate_b, sw1, sb1, sw2, sb2, rw1, rb1, rw2, rb2):
    x = np.asarray(x, np.float32)
    gate_w = np.asarray(gate_w, np.float32)
    gate_b = np.asarray(gate_b, np.float32)
    sw1 = np.asarray(sw1, np.float32)
    sb1 = np.asarray(sb1, np.float32)
    sw2 = np.asarray(sw2, np.float32)
    sb2 = np.asarray(sb2, np.float32)
    rw1 = np.asarray(rw1, np.float32)
    rb1 = np.asarray(rb1, np.float32)
    rw2 = np.asarray(rw2, np.float32)
    rb2 = np.asarray(rb2, np.float32)

    t = x.reshape(T, D)

    # ---- router on host (part of the dispatch/sharding step) ----
    logits = t @ gate_w.T + gate_b
    m = logits.max(axis=1, keepdims=True)
    ex = np.exp(logits - m)
    probs = ex / ex.sum(axis=1, keepdims=True)
    top_i = np.argpartition(-probs, K - 1, axis=1)[:, :K]          # [T, K]

    sel = np.zeros((T, E), bool)
    sel[np.arange(T)[:, None], top_i] = True
    idxs = [np.nonzero(sel[:, e])[0] for e in range(E)]
    counts = np.array([len(i) for i in idxs])
    cpad = max(CHUNK, int(-(-counts.max() // 16) * 16))

    if cpad not in _cache:
        _cache[cpad] = _build(cpad)
    nc = _cache[cpad]

    # ---- shared-expert weights: fp8 hi/16/lo trio of w1 (scaled), fp16 w2 ----
    w1c = sw1.reshape(FS, D)                       # [FS, D]
    w1h = _q8(WSCALE * w1c)
    w1h16 = _q8(RSCALE * w1c)
    w1lo = _q8(WSCALE * w1c - w1h.astype(np.float32))
    # pack each copy [128, KD, FS] then interleave to [g][copy][k][WG*128]
    def pk_w1(a):                                  # -> [128, KD, FS]
        return _pack_k(np.ascontiguousarray(a.T)).reshape(128, KD, FS)
    trio = np.stack([pk_w1(w1h), pk_w1(w1h16), pk_w1(w1lo)], axis=1)
    # [128, 3, KD, FS] -> [128, g, 3, KD, WG*128]
    trio = trio.reshape(128, 3, KD, FSH // WG, WG * 128).transpose(0, 3, 1, 2, 4)
    sw1p = np.ascontiguousarray(trio.reshape(128, -1))

    w2c = np.concatenate([sw2[0], sw2[1]], axis=1)  # [D, FS]
    sw2p = _pack_k(np.ascontiguousarray(w2c.T).astype(np_f16))
    sb1c = np.ascontiguousarray(sb1.reshape(FSH, 128).T)

    # shared tokens: xh and rx16 = e4m3(16*(x - xh)), both k-packed
    in_maps = []
    for c in range(N_CORES):
        tc_ = t[c * TS:(c + 1) * TS]               # [TS, D]
        xh = _q8(tc_)
        rx16 = _q8(RSCALE * (tc_ - xh.astype(np.float32)))
        xsp = np.concatenate(
            [_pack_k(np.ascontiguousarray(xh.T)),
             _pack_k(np.ascontiguousarray(rx16.T))], axis=1)

        idx = idxs[c]
        ce = len(idx)
        sizes_r = _routed_sizes(cpad)
        segA = sizes_r[0]
        xgT = np.zeros((D, cpad), np_f8)
        xgT[:, :ce] = _q8(t[idx]).T
        xgp = np.concatenate(
            [_pack_k(np.ascontiguousarray(xgT[:, :segA])),
             _pack_k(np.ascontiguousarray(xgT[:, segA:]))], axis=1)
        cwv = np.zeros((128, cpad), np_f16)
        cwv[:, :ce] = (probs[idx, c] / WSCALE)[None, :].astype(np_f16)

        # rw1: [F, D] -> k-pack -> regroup to [g(4)][k(4)][fcol(512)]
        rw1k = _pack_k(np.ascontiguousarray(_q8(WSCALE * rw1[c]).T))
        rw1p = np.ascontiguousarray(
            rw1k.reshape(128, KD, 4, 512).transpose(0, 2, 1, 3)
            .reshape(128, -1))
        rw2p = _pack_k(np.ascontiguousarray(_q8(WSCALE * rw2[c]).T))
        in_maps.append({
            "xg": xgp,
            "cwb": cwv,
            "rw": np.concatenate([rw1p, rw2p], axis=1),
            "rb1": np.ascontiguousarray(rb1[c].reshape(FR, 128).T),
            "xs": xsp,
            "sw1": sw1p,
            "sw2": sw2p,
            "sb1": sb1c,
        })

    res = run_bass_kernel_spmd(nc, in_maps, core_ids=list(range(N_CORES)))

    # ---- combine on host ----
    out = np.empty((T, D), np.float32)
    for c in range(N_CORES):
        out[c * TS:(c + 1) * TS] = res.results[c]["ys"].T.astype(np.float32)
    for c in range(N_CORES):
        idx = idxs[c]
        out[idx] += res.results[c]["yr"][:, :len(idx)].T.astype(np.float32)

    # output biases (zero in the spec, handled exactly anyway)
    if sb2.any() or rb2.any():
        cw = np.zeros((T, E), np.float32)
        np.add.at(cw, (np.arange(T)[:, None], top_i),
                  np.take_along_axis(probs, top_i, axis=1))
        out += sb2.sum(axis=0)[None, :] + cw @ rb2

    return out.reshape(B, S, D)


# revision 22
# speedup vs baseline: 1.5082x; 1.1209x over previous
"""DeepseekMoE kernel for 8 Trainium2 NeuronCores.

Strategy (expert-parallel routed + data-parallel shared, fp8 DoubleRow):
  - Host computes the router (gate matmul, softmax, top-2) in numpy and
    gathers each expert's tokens (classic MoE dispatch, done host-side as
    part of sharding).
  - Core c runs routed expert c's FFN over its gathered tokens (padded to
    a common Cpad, multiple of 16, so all 8 cores run the same SPMD
    program), scaling the output by the combine weights on-device (DVE).
  - Shared experts' weights are replicated; each core runs them over a
    distinct 512-token slice of the batch (data-parallel).
  - Matmul precision plan (PSUM always accumulates fp32):
      * routed L1/L2: fp8 e4m3 operands with perf_mode=DoubleRow
        (2 fp8 weights/PE cell -> 2 k-tiles contracted per instruction).
        The routed output is scaled by top-2 softmax weights (~0.17 norm
        share of the final output), so 1-term fp8 error (~5% rel) only
        contributes ~0.9% to the result.
      * shared L1: 3-term residual-corrected fp8 DoubleRow:
          256*(x@w1) = xh@W1h + rx16@W1h16 + xh@W1lo
        with xh=e4m3(x), rx16=e4m3(16*(x-xh)), W1h=e4m3(256*w1),
        W1h16=e4m3(16*w1), W1lo=e4m3(256*w1-W1h). Host packs all copies.
        The fp8 weight scale (256x) centers N(0, 0.02) weights in e4m3's
        normal range; it is undone by the GELU activation's input scale.
      * shared L2: plain fp16 (h written fp16 by ACT; w2 stored fp16) --
        the shared output carries ~98% of the result norm, so its h/w2
        cannot take a single-quantization fp8 error.
  - Layout is fully transposed (features on partitions, tokens on the
    free dim), k-blocks of the contraction as a middle AP dim so
    DoubleRow's [128, 2, n] operand shape falls out of a plain slice.
  - Input DMAs are a few large transfers (>=512B contiguous runs for
    full modeled DMA bandwidth), issued in consumption order and spread
    across the SP and DVE sequencers (each dma_start costs ~1.2us of
    sequencer time); outputs ride the SWDGE (gpsimd) path.
"""

import numpy as np
import ml_dtypes

import concourse.bass as bass
import concourse.tile as tile
import concourse.mybir as mybir
from concourse import bacc
from concourse.bass_utils import run_bass_kernel_spmd

B, S, D, F, E, NS, K = 2, 2048, 512, 2048, 8, 2, 2
T = B * S
N_CORES = 8
TS = T // N_CORES          # shared-expert tokens per core
FS = NS * F                # concatenated shared FFN width
CHUNK = 512                # token chunk (= max fp32 PSUM bank free dim)
KD = D // 128              # 4  k-tiles over D
FR = F // 128              # 16 f-tiles routed
FSH = FS // 128            # 32 f-tiles shared
DD = D // 128              # 4  output d-tiles
WG = 4                     # f-tiles per shared weight-group DMA

FP8 = mybir.dt.float8e4
F16 = mybir.dt.float16
F32 = mybir.dt.float32
np_f8 = ml_dtypes.float8_e4m3
np_f16 = np.float16

_GELU = mybir.ActivationFunctionType.Gelu
_DR = mybir.MatmulPerfMode.DoubleRow
WSCALE = 256.0
RSCALE = 16.0

_cache: dict = {}


def _routed_sizes(cpad):
    """Token-chunk sizes for the routed phase: a moderate head chunk (fast
    PE start -- less DMA to wait for), 512s in the middle, 256 reserved for
    the tail chunk that runs AFTER the shared phase (short kernel drain).
    All sizes multiples of 16 (DoubleRow AP step constraint)."""
    if cpad <= CHUNK:
        return [cpad]
    head = cpad - 256 - ((cpad - 512) // CHUNK) * CHUNK
    sizes = [head] if head else []
    sizes += [CHUNK] * ((cpad - 256 - head) // CHUNK)
    return sizes + [256]


def _chunk_offsets(total, sizes):
    out, c0 = [], 0
    for s in sizes:
        out.append((c0, s))
        c0 += s
    return out


def _build(cpad: int):
    nc = bacc.Bacc("TRN2", debug=False)

    sizes_r = _routed_sizes(cpad)
    segA = sizes_r[0]            # first chunk: own k-major segment
    segB = cpad - segA           # remaining chunks share one segment
    chunks_r = _chunk_offsets(cpad, sizes_r)

    xg = nc.dram_tensor("xg", [128, KD * cpad], FP8, kind="ExternalInput")
    cwb = nc.dram_tensor("cwb", [128, cpad], F16, kind="ExternalInput")
    rw = nc.dram_tensor("rw", [128, KD * F + FR * D], FP8, kind="ExternalInput")
    rb1 = nc.dram_tensor("rb1", [128, FR], F32, kind="ExternalInput")
    xs = nc.dram_tensor("xs", [128, 2 * KD * TS], FP8, kind="ExternalInput")
    sw1 = nc.dram_tensor("sw1", [128, 2 * KD * FS], FP8, kind="ExternalInput")
    sw2 = nc.dram_tensor("sw2", [128, FSH * D], F16, kind="ExternalInput")
    sb1 = nc.dram_tensor("sb1", [128, FSH], F32, kind="ExternalInput")
    yr = nc.dram_tensor("yr", [D, cpad], F16, kind="ExternalOutput")
    ys = nc.dram_tensor("ys", [D, TS], F16, kind="ExternalOutput")

    # sw1 free-dim layout: [group g(8)][copy t(2)][k(4)][fcol(WG*128)]
    NCOPY = 2                    # {W1h, W1lo}; x-residual pairs with W1h
    GCOL = NCOPY * KD * WG * 128  # columns per group block

    with tile.TileContext(nc) as tc:
        with (
            tc.tile_pool(name="wts", bufs=1) as wts,
            tc.tile_pool(name="acts", bufs=1) as acts,
            tc.tile_pool(name="hp", bufs=4) as hp,
            tc.tile_pool(name="op", bufs=3) as op,
            tc.tile_pool(name="ps1", bufs=4, space="PSUM") as ps1,
            tc.tile_pool(name="ps2", bufs=1, space="PSUM") as ps2,
        ):
            # ---- t=0 warmup while the first DMAs are in flight: trigger the
            # GELU ACT-table load now (~1.3 us on first use), and run dummy
            # matmuls so the PE p-state is at full clock for the first real
            # matmul ----
            warm = wts.tile([128, 1024], FP8, name="warm_in")
            nc.vector.memset(warm[:], 0.0)
            wb = wts.tile([128, 1], F32, name="warm_b")
            nc.vector.memset(wb[:], 0.0)
            wh = hp.tile([128, 512], FP8, name="wh")
            nc.scalar.activation(wh[:], warm[:, 0:512], _GELU, bias=wb[:])
            wp = ps1.tile([128, 512], F32, tag="p1", name="warmp")
            w3 = warm.rearrange("p (two c) -> p two c", two=2)
            # enough back-to-back matmuls (~3us) that the p-state ramp
            # completes right as the first real chunk's data lands
            for _ in range(14):
                nc.tensor.matmul(wp[:], w3[:, :, 0:128], w3[:],
                                 start=True, stop=True, perf_mode=_DR)

            # ---- resident SBUF images of all inputs ----
            xg_sb = acts.tile([128, KD * cpad], FP8, name="xg_sb")
            rw_sb = wts.tile([128, KD * F + FR * D], FP8, name="rw_sb")
            rb1_sb = wts.tile([128, FR], F32, name="rb1_sb")
            cw_sb = acts.tile([128, cpad], F16, name="cw_sb")
            xs_sb = acts.tile([128, 2 * KD * TS], FP8, name="xs_sb")
            sw1_sb = wts.tile([128, NCOPY * KD * FS], FP8, name="sw1_sb")
            sw2_sb = wts.tile([128, FSH * D], F16, name="sw2_sb")
            sb1_sb = wts.tile([128, FSH], F32, name="sb1_sb")

            # ---- preload: ALL inputs on the single SP/HWDGE queue in strict
            # consumption order -- one queue means one global wire order, so
            # later (shared) transfers can never cut in front of the routed
            # phase's inputs on the serialized DMA wire. Outputs ride SWDGE.
            RW1, RWQ = KD * F, (KD * F) // 4
            nc.sync.dma_start(xg_sb[:, 0:KD * segA], xg.ap()[:, 0:KD * segA])
            # routed weights in 4 interleaved f-major quarters (w1 f-quarter
            # + w2 f-quarter); both packed f-group-major so each quarter is
            # one contiguous transfer in consumption order
            for q in range(4):
                nc.sync.dma_start(
                    rw_sb[:, q * RWQ:(q + 1) * RWQ],
                    rw.ap()[:, q * RWQ:(q + 1) * RWQ])
                if q == 0:
                    nc.sync.dma_start(rb1_sb[:], rb1.ap())
                nc.sync.dma_start(
                    rw_sb[:, RW1 + q * RWQ:RW1 + (q + 1) * RWQ],
                    rw.ap()[:, RW1 + q * RWQ:RW1 + (q + 1) * RWQ])
            nc.sync.dma_start(xg_sb[:, KD * segA:], xg.ap()[:, KD * segA:])
            nc.sync.dma_start(cw_sb[:], cwb.ap())
            nc.sync.dma_start(xs_sb[:], xs.ap())
            nc.sync.dma_start(sb1_sb[:], sb1.ap())
            for g in range(FSH // WG):
                nc.sync.dma_start(sw1_sb[:, g * GCOL:(g + 1) * GCOL],
                                  sw1.ap()[:, g * GCOL:(g + 1) * GCOL])
                nc.sync.dma_start(
                    sw2_sb[:, g * WG * D:(g + 1) * WG * D],
                    sw2.ap()[:, g * WG * D:(g + 1) * WG * D])

            # ---- AP helpers ----
            xg3A = xg_sb[:, 0:KD * segA].rearrange("p (k c) -> p k c", k=KD)
            xg3B = xg_sb[:, KD * segA:].rearrange("p (k c) -> p k c", k=KD)
            # rw1 packed f-group-major: [g(4)][k(4)][fcol(512)]
            rw13 = rw_sb[:, 0:RW1].rearrange("p (g k f) -> p g k f", g=4, k=KD)
            rw23 = rw_sb[:, RW1:].rearrange("p (f d) -> p f d", f=FR)
            xs3 = xs_sb.rearrange("p (t k c) -> p t k c", t=2, k=KD)
            sw13 = sw1_sb.rearrange("p (g t k f) -> p g t k f", g=FSH // WG,
                                    t=NCOPY, k=KD)
            sw23 = sw2_sb.rearrange("p (f d) -> p f d", f=FSH)

            def x_route(i, kk, cs):
                if i == 0:
                    return xg3A[:, 2 * kk:2 * kk + 2, 0:cs]
                c0 = chunks_r[i][0] - segA
                return xg3B[:, 2 * kk:2 * kk + 2, c0:c0 + cs]

            # ---- chunk descriptors ----
            def r_chunk(i, c0, cs):
                return dict(
                    kind="r", cs=cs, c0=c0, nf=FR,
                    x=lambda kk, i=i, cs=cs: x_route(i, kk, cs),
                    w1=lambda kk, f: rw13[:, f // 4, 2 * kk:2 * kk + 2,
                                          (f % 4) * 128:(f % 4 + 1) * 128],
                    w2=lambda fp, d: rw23[:, 2 * fp:2 * fp + 2,
                                          d * 128:(d + 1) * 128],
                )

            def s_chunk(c0, cs):
                return dict(
                    kind="s", cs=cs, c0=c0, nf=FSH,
                    x=lambda t, kk, c0=c0, cs=cs:
                        xs3[:, t, 2 * kk:2 * kk + 2, c0:c0 + cs],
                    w1=lambda t, kk, f: sw13[:, f // WG, t, 2 * kk:2 * kk + 2,
                                             (f % WG) * 128:
                                             (f % WG + 1) * 128],
                    w2=lambda f, d: sw23[:, f, d * 128:(d + 1) * 128],
                )

            routed_chunks = [r_chunk(i, c0, cs)
                             for i, (c0, cs) in enumerate(chunks_r)]
            shared_chunks = [s_chunk(c0, cs)
                             for c0, cs in _chunk_offsets(TS, [CHUNK] * (TS // CHUNK))]
            # the last (small) routed chunk runs AFTER the shared phase: its
            # inputs are long resident and its evacuation+DMA tail is short,
            # and the shared phase's larger output drain overlaps its compute
            chunks = routed_chunks[:-1] + shared_chunks + routed_chunks[-1:]
            steps = [(ch, f) for ch in chunks for f in range(ch["nf"])]

            # ---- software-pipelined emission: the f-tile's first-layer
            # matmuls run LOOKAHEAD steps ahead of the second-layer matmuls
            # that consume its activation output ----
            LOOKAHEAD = 2
            h_tiles: dict = {}
            po_tiles: dict = {}
            for i in range(len(steps) + LOOKAHEAD):
                if i < len(steps):
                    ch, f = steps[i]
                    cs = ch["cs"]
                    # PSUM tiles are allocated bank-sized (512 fp32) and
                    # prefix-sliced so a partial chunk never crosses a bank
                    p1 = ps1.tile([128, CHUNK], F32, name="p1")[:, 0:cs]
                    if ch["kind"] == "r":
                        for kk in range(KD // 2):
                            nc.tensor.matmul(
                                p1[:], ch["w1"](kk, f), ch["x"](kk),
                                start=(kk == 0), stop=(kk == KD // 2 - 1),
                                perf_mode=_DR)
                        # routed h tiles come in pairs: DoubleRow L2 consumes
                        # [128, 2, cs] (two f-tiles) per instruction
                        if f % 2 == 0:
                            hpair = hp.tile([128, 2 * cs], FP8, name="h")
                            h_tiles[i] = hpair
                        else:
                            hpair = h_tiles[i - 1]
                            h_tiles[i] = hpair
                        nc.scalar.activation(
                            hpair[:, (f % 2) * cs:(f % 2 + 1) * cs], p1[:],
                            _GELU, bias=rb1_sb[:, f:f + 1], scale=1.0 / WSCALE)
                    else:
                        # 3 terms: xh@W1h + rx@W1h + xh@W1lo (rx is the
                        # direct e4m3 residual of x, so it pairs with W1h)
                        terms = [(0, 0), (1, 0), (0, 1)]
                        for ti, (tx, tw) in enumerate(terms):
                            for kk in range(KD // 2):
                                nc.tensor.matmul(
                                    p1[:], ch["w1"](tw, kk, f),
                                    ch["x"](tx, kk),
                                    start=(ti == 0 and kk == 0),
                                    stop=(ti == 2 and kk == KD // 2 - 1),
                                    perf_mode=_DR)
                        h = hp.tile([128, cs], F16, name="hs")
                        nc.scalar.activation(h[:], p1[:], _GELU,
                                             bias=sb1_sb[:, f:f + 1],
                                             scale=1.0 / WSCALE)
                        h_tiles[i] = h
                j = i - LOOKAHEAD
                if j >= 0:
                    ch, f = steps[j]
                    cs, c0 = ch["cs"], ch["c0"]
                    routed = ch["kind"] == "r"
                    if f == 0:
                        po_tiles[id(ch)] = [
                            ps2.tile([128, CHUNK], F32, tag=f"o{d}",
                                     name=f"po{d}")[:, 0:cs]
                            for d in range(DD)
                        ]
                    po = po_tiles[id(ch)]
                    h = h_tiles.pop(j)
                    if routed:
                        if f % 2 == 1:      # consume the completed h pair
                            h3 = h.rearrange("p (two c) -> p two c", two=2)
                            for d in range(DD):
                                nc.tensor.matmul(
                                    po[d][:], ch["w2"](f // 2, d), h3[:],
                                    start=(f == 1), stop=(f == ch["nf"] - 1),
                                    perf_mode=_DR)
                    else:
                        for d in range(DD):
                            nc.tensor.matmul(
                                po[d][:], ch["w2"](f, d), h[:],
                                start=(f == 0), stop=(f == ch["nf"] - 1))
                    if f == ch["nf"] - 1:
                        o = op.tile([128, DD * cs], F16, name="o")
                        last = ch is chunks[-1]
                        for d in range(DD):
                            if routed:
                                nc.vector.tensor_mul(
                                    o[:, d * cs:(d + 1) * cs], po[d][:],
                                    cw_sb[:, c0:c0 + cs])
                            else:
                                nc.vector.tensor_copy(
                                    o[:, d * cs:(d + 1) * cs], po[d][:])
                        # one wide DMA per chunk; SWDGE path (separate FIFO
                        # from the input preload). The final chunk rides
                        # HWDGE (lower latency; preload long finished).
                        ydst = (yr if routed else ys).ap().rearrange(
                            "(dd p) c -> p dd c", p=128)[:, :, c0:c0 + cs]
                        ysrc = o.rearrange("p (dd c) -> p dd c", dd=DD)
                        if last:
                            nc.sync.dma_start(ydst, ysrc)
                        else:
                            nc.gpsimd.dma_start(ydst, ysrc)
                        del po_tiles[id(ch)]

    nc.compile()
    return nc


def _q8(a):
    return np.asarray(a, np.float32).astype(np_f8)


def _pack_k(a2d):
    """[K*128, N] -> [128, K*N] with k-blocks along the free dim."""
    k = a2d.shape[0] // 128
    return np.ascontiguousarray(
        a2d.reshape(k, 128, -1).transpose(1, 0, 2).reshape(128, -1))


def kernel(x, gate_w, gate_b, sw1, sb1, sw2, sb2, rw1, rb1, rw2, rb2):
    x = np.asarray(x, np.float32)
    gate_w = np.asarray(gate_w, np.float32)
    gate_b = np.asarray(gate_b, np.float32)
    sw1 = np.asarray(sw1, np.float32)
    sb1 = np.asarray(sb1, np.float32)
    sw2 = np.asarray(sw2, np.float32)
    sb2 = np.asarray(sb2, np.float32)
    rw1 = np.asarray(rw1, np.float32)
    rb1 = np.asarray(rb1, np.float32)
    rw2 = np.asarray(rw2, np.float32)
    rb2 = np.asarray(rb2, np.float32)

    t = x.reshape(T, D)

    # ---- router on host (part of the dispatch/sharding step) ----
    logits = t @ gate_w.T + gate_b
    m = logits.max(axis=1, keepdims=True)
    ex = np.exp(logits - m)
    probs = ex / ex.sum(axis=1, keepdims=True)
    top_i = np.argpartition(-probs, K - 1, axis=1)[:, :K]          # [T, K]

    sel = np.zeros((T, E), bool)
    sel[np.arange(T)[:, None], top_i] = True
    idxs = [np.nonzero(sel[:, e])[0] for e in range(E)]
    counts = np.array([len(i) for i in idxs])
    cpad = max(CHUNK, int(-(-counts.max() // 16) * 16))

    if cpad not in _cache:
        _cache[cpad] = _build(cpad)
    nc = _cache[cpad]

    # ---- shared-expert weights: fp8 {hi, lo} duo of w1 (scaled), fp16 w2 ----
    w1c = sw1.reshape(FS, D)                       # [FS, D]
    w1h = _q8(WSCALE * w1c)
    w1lo = _q8(WSCALE * w1c - w1h.astype(np.float32))
    # pack each copy [128, KD, FS] then interleave to [g][copy][k][WG*128]
    def pk_w1(a):                                  # -> [128, KD, FS]
        return _pack_k(np.ascontiguousarray(a.T)).reshape(128, KD, FS)
    duo = np.stack([pk_w1(w1h), pk_w1(w1lo)], axis=1)
    # [128, 2, KD, FS] -> [128, g, 2, KD, WG*128]
    duo = duo.reshape(128, 2, KD, FSH // WG, WG * 128).transpose(0, 3, 1, 2, 4)
    sw1p = np.ascontiguousarray(duo.reshape(128, -1))

    w2c = np.concatenate([sw2[0], sw2[1]], axis=1)  # [D, FS]
    sw2p = _pack_k(np.ascontiguousarray(w2c.T).astype(np_f16))
    sb1c = np.ascontiguousarray(sb1.reshape(FSH, 128).T)

    # shared tokens: xh and the direct e4m3 residual rx = e4m3(x - xh)
    # (small values land in e4m3's subnormal range; they pair with the
    # already-256x-scaled W1h so no extra weight copy is needed)
    in_maps = []
    for c in range(N_CORES):
        tc_ = t[c * TS:(c + 1) * TS]               # [TS, D]
        xh = _q8(tc_)
        rx = _q8(tc_ - xh.astype(np.float32))
        xsp = np.concatenate(
            [_pack_k(np.ascontiguousarray(xh.T)),
             _pack_k(np.ascontiguousarray(rx.T))], axis=1)

        idx = idxs[c]
        ce = len(idx)
        sizes_r = _routed_sizes(cpad)
        segA = sizes_r[0]
        xgT = np.zeros((D, cpad), np_f8)
        xgT[:, :ce] = _q8(t[idx]).T
        xgp = np.concatenate(
            [_pack_k(np.ascontiguousarray(xgT[:, :segA])),
             _pack_k(np.ascontiguousarray(xgT[:, segA:]))], axis=1)
        cwv = np.zeros((128, cpad), np_f16)
        cwv[:, :ce] = (probs[idx, c] / WSCALE)[None, :].astype(np_f16)

        # rw1: [F, D] -> k-pack -> regroup to [g(4)][k(4)][fcol(512)]
        rw1k = _pack_k(np.ascontiguousarray(_q8(WSCALE * rw1[c]).T))
        rw1p = np.ascontiguousarray(
            rw1k.reshape(128, KD, 4, 512).transpose(0, 2, 1, 3)
            .reshape(128, -1))
        rw2p = _pack_k(np.ascontiguousarray(_q8(WSCALE * rw2[c]).T))
        in_maps.append({
            "xg": xgp,
            "cwb": cwv,
            "rw": np.concatenate([rw1p, rw2p], axis=1),
            "rb1": np.ascontiguousarray(rb1[c].reshape(FR, 128).T),
            "xs": xsp,
            "sw1": sw1p,
            "sw2": sw2p,
            "sb1": sb1c,
        })

    res = run_bass_kernel_spmd(nc, in_maps, core_ids=list(range(N_CORES)))

    # ---- combine on host ----
    out = np.empty((T, D), np.float32)
    for c in range(N_CORES):
        out[c * TS:(c + 1) * TS] = res.results[c]["ys"].T.astype(np.float32)
    for c in range(N_CORES):
        idx = idxs[c]
        out[idx] += res.results[c]["yr"][:, :len(idx)].T.astype(np.float32)

    # output biases (zero in the spec, handled exactly anyway)
    if sb2.any() or rb2.any():
        cw = np.zeros((T, E), np.float32)
        np.add.at(cw, (np.arange(T)[:, None], top_i),
                  np.take_along_axis(probs, top_i, axis=1))
        out += sb2.sum(axis=0)[None, :] + cw @ rb2

    return out.reshape(B, S, D)
